# revision 9
# baseline (speedup 1.0000x reference)
"""Exponential smoothing (per-channel EMA over time) on 8 Trainium2 cores.

  s_0 = x_0 ; s_t = a * x_t + (1 - a) * s_{t-1},  a = sigmoid(alpha)  (per channel)

Full shapes: x (16, 4096, 512) f32, alpha (1, 1, 512) f32 -> out (16, 4096, 512).

Design (trace-driven):
  * All HBM I/O is fp16, time-major per core, with each TC-chunk stored as
    blocked halves [even timesteps | odd timesteps]: the host preps the
    layout and upcasts/interleaves the result (pure layout transforms; the
    2e-2 global-rel-err budget dwarfs fp16's 2^-11 and the EMA is a convex
    combination). Halves DMA bytes, removes on-device transposes, and makes
    every on-device operand contiguous (the DVE 2x/4x packed fast modes
    require stride-1 2-byte operands). The per-channel parameter transforms
    (a, w, w^2, 1/a -- 512 elements) are host-precomputed into one (128, 16)
    f32 tile.
  * The hardware scan (TensorTensorScanArith, vector engine) costs ~2.13
    ns/elem/lane regardless of dtype -- the dominant cost. So the kernel
    scans ONLY the odd timesteps (an EMA with decay w^2 over combined
    inputs g_i = w*x_{2i} + x_{2i+1}):
      - combine g: tensor engine, diag(w) @ x_even + I @ x_odd accumulated
        into PSUM (the scan reads data1 straight from PSUM). Combines are
        issued one chunk ahead of the fill and g triple-buffers so the PE
        stream stalls as little as possible (PE clock ramps 1.2 -> 2.4 GHz
        only after ~3 us of continuous execution).
      - odd scan: r_{2i+1} = w^2 * r_{2i-1} + g_i on the vector engine,
        scanning r = s/a (initial r_{-1} = x_0/a) so raw x is the scan
        input. The scan writes cols 1..NH of an [128, NH+1] tile whose col
        0 holds the initial; the fill's shifted operand is then contiguous.
      - even fill: r_{2i} = w * r_{2i-1} + x_{2i}. Load-balanced: most
        chunks on the tensor engine (diag(w) @ r_shift + I @ x_even into
        PSUM, evacuated by the scalar engine), every third chunk on the
        vector engine (one packed scalar_tensor_tensor into SBUF).
      - evacuate+scale y = a * r: odd halves always on the vector engine
        (packed fp16 tensor_scalar ~0.2 ns/elem); even halves on the scalar
        engine (PSUM source) or vector engine (SBUF source). gpsimd
        elementwise ops crash the Q7 handler -- do not use.
  * Loads ride the SP hardware-DGE queue; stores ride the Scalar engine's
    hardware-DGE queue (the gpsimd software-DGE path sustains only ~170
    GB/s). Both spread across all 16 DMA engines.
"""

from contextlib import ExitStack

import numpy as np

import concourse.tile as tile
from concourse import bacc, mybir
from concourse.bass_utils import run_bass_kernel_spmd
from concourse.masks import make_identity

B, T, D = 16, 4096, 512
NCORES = 8
BL = B // NCORES   # batches per core
P = 128            # partitions
TC = 2048          # time chunk per pipeline step
NH = TC // 2       # odd (= even) timesteps per chunk
ND = D // P        # channel chunks of 128
MM = 512           # max moving free dim per matmul
NTC = T // TC
DVE_FILL_EVERY = 3  # chunks c with c % 3 == 2 fill on the vector engine

FP32 = mybir.dt.float32
FP16 = mybir.dt.float16


def build_program(bl: int = BL, t: int = T) -> bacc.Bacc:
    """Build the per-core Bass program (same NEFF for all 8 cores)."""
    ntc = t // TC
    nc = bacc.Bacc(
        "TRN2",
        target_bir_lowering=False,
        debug=False,
        enable_asserts=False,
        num_devices=NCORES,
    )
    x = nc.dram_tensor("xt", (bl, D, t), FP16, kind="ExternalInput").ap()
    # Host-precomputed per-channel coefficients, partition-major:
    # col q*ND + j = quantity q for channel chunk j (q: 0=a, 1=w, 2=w^2, 3=1/a)
    coef = nc.dram_tensor("coef", (P, 4 * ND), FP32, kind="ExternalInput").ap()
    y = nc.dram_tensor("yt", (bl, D, t), FP16, kind="ExternalOutput").ap()

    with tile.TileContext(nc) as tc, ExitStack() as ctx:
        const_pool = ctx.enter_context(tc.tile_pool(name="const", bufs=1))
        x_pool = ctx.enter_context(tc.tile_pool(name="x", bufs=6))
        g_pool = ctx.enter_context(tc.tile_pool(name="g", bufs=3, space="PSUM"))
        rep_pool = ctx.enter_context(tc.tile_pool(name="rep", bufs=1, space="PSUM"))
        res_pool = ctx.enter_context(tc.tile_pool(name="res", bufs=3))
        r_pool = ctx.enter_context(tc.tile_pool(name="r", bufs=10))
        y_pool = ctx.enter_context(tc.tile_pool(name="y", bufs=4))

        # Identity first: gpsimd builds it while the coef DMA runs.
        ident = const_pool.tile([P, P], FP16)
        make_identity(nc, ident[:])

        coef_sb = const_pool.tile([P, 4 * ND], FP32)
        nc.sync.dma_start(coef_sb[:], coef[:, :])
        a_sb = coef_sb[:, 0 * ND : 1 * ND]
        w_sb = coef_sb[:, 1 * ND : 2 * ND]
        w2_sb = coef_sb[:, 2 * ND : 3 * ND]
        inv_a = coef_sb[:, 3 * ND : 4 * ND]

        diag_w = []
        for j in range(ND):
            dw = const_pool.tile([P, P], FP16, tag=f"dw{j}")
            nc.vector.tensor_scalar_mul(dw[:], ident[:], w_sb[:, j : j + 1])
            diag_w.append(dw)

        # Scan data0: w^2 broadcast along the time axis (full packed operand).
        ones = const_pool.tile([P, NH], FP16)
        nc.vector.memset(ones[:], 1.0)
        w2b = []
        for j in range(ND):
            wt = const_pool.tile([P, NH], FP16, tag=f"w2b{j}")
            nc.scalar.mul(wt[:], ones[:], w2_sb[:, j : j + 1])
            w2b.append(wt)

        # 8 chains (2 batches x 4 channel chunks), ntc chunks each, chunk
        # order: all chunk-0s, then all chunk-1s (carries ready early).
        chunks = [
            (tci, b, j) for tci in range(ntc) for b in range(bl) for j in range(ND)
        ]
        n = len(chunks)
        xcs, gps = [None] * n, [None] * n
        r_prev = [[None] * ND for _ in range(bl)]

        def load_and_combine(c):
            tci, b, j = chunks[c]
            t0 = tci * TC
            xc = x_pool.tile([P, TC], FP16, tag="x")  # [evens | odds] blocked
            nc.sync.dma_start(xc[:], x[b, j * P : (j + 1) * P, t0 : t0 + TC])
            xe, xo = xc[:, 0:NH], xc[:, NH:TC]
            # g_i = w * x_{2i} + x_{2i+1}: per-stationary batched matmuls.
            gp = g_pool.tile([P, NH], FP32, tag="g")
            for h in range(NH // MM):
                c0, c1 = h * MM, (h + 1) * MM
                nc.tensor.matmul(
                    gp[:, c0:c1], diag_w[j][:], xe[:, c0:c1], start=True, stop=False
                )
            for h in range(NH // MM):
                c0, c1 = h * MM, (h + 1) * MM
                nc.tensor.matmul(
                    gp[:, c0:c1], ident[:], xo[:, c0:c1], start=False, stop=True
                )
            xcs[c], gps[c] = xc, gp

        load_and_combine(0)
        load_and_combine(1)
        for c in range(n):
            tci, b, j = chunks[c]
            t0 = tci * TC
            xc, gp = xcs[c], gps[c]
            xe = xc[:, 0:NH]

            # ro col 0 = initial r_{-1}; cols 1..NH = scan of odds.
            ro = r_pool.tile([P, NH + 1], FP16, tag="r")
            if tci == 0:
                nc.vector.tensor_scalar_mul(
                    ro[:, 0:1], xc[:, 0:1], inv_a[:, j : j + 1]
                )
            else:
                nc.vector.tensor_copy(ro[:, 0:1], r_prev[b][j][:, NH : NH + 1])
            nc.vector.tensor_tensor_scan(
                ro[:, 1 : NH + 1],
                w2b[j][:],
                gp[:],
                ro[:, 0:1],
                mybir.AluOpType.mult,
                mybir.AluOpType.add,
            )
            r_prev[b][j] = ro

            # Next chunk's combine goes on the PE queue BEFORE this fill.
            if c + 2 < n:
                load_and_combine(c + 2)

            # r_{2i} = w * r_{2i-1} + x_{2i}, then y = a * r into blocked
            # [evens | odds]; host de-interleaves.
            yc = y_pool.tile([P, TC], FP16, tag="y")
            if c % DVE_FILL_EVERY == DVE_FILL_EVERY - 1:
                # Fill + both evacs on the vector engine (all packed fp16).
                rs = res_pool.tile([P, NH], FP16, tag="res")
                nc.vector.scalar_tensor_tensor(
                    rs[:], ro[:, 0:NH], w_sb[:, j : j + 1], xe,
                    mybir.AluOpType.mult, mybir.AluOpType.add,
                )
                nc.vector.tensor_scalar_mul(yc[:, 0:NH], rs[:], a_sb[:, j : j + 1])
            else:
                rep = rep_pool.tile([P, NH], FP32, tag="rep")
                for h in range(NH // MM):
                    c0, c1 = h * MM, (h + 1) * MM
                    nc.tensor.matmul(
                        rep[:, c0:c1], diag_w[j][:], ro[:, c0:c1],
                        start=True, stop=False,
                    )
                for h in range(NH // MM):
                    c0, c1 = h * MM, (h + 1) * MM
                    nc.tensor.matmul(
                        rep[:, c0:c1], ident[:], xe[:, c0:c1],
                        start=False, stop=True,
                    )
                nc.scalar.mul(yc[:, 0:NH], rep[:], a_sb[:, j : j + 1])
            nc.vector.tensor_scalar_mul(
                yc[:, NH:TC], ro[:, 1 : NH + 1], a_sb[:, j : j + 1]
            )
            nc.scalar.dma_start(y[b, j * P : (j + 1) * P, t0 : t0 + TC], yc[:])

    nc.compile()
    return nc


_prog = None


def _get_prog():
    global _prog
    if _prog is None:
        _prog = build_program()
    return _prog


def make_coef(alpha):
    """Precompute per-channel (a, w, w^2, 1/a) packed as (128, 4*ND) f32."""
    al = np.asarray(alpha, dtype=np.float64).reshape(D)
    a = 1.0 / (1.0 + np.exp(-al))
    w = 1.0 - a
    quants = [a, w, w * w, 1.0 / a]
    out = np.empty((P, 4 * ND), dtype=np.float32)
    for q, v in enumerate(quants):
        # channel d = j*128 + p -> column q*ND + j, row p
        out[:, q * ND : (q + 1) * ND] = v.reshape(ND, P).T
    return out


def make_in_maps(x, alpha):
    """Per-core inputs: blocked-even/odd time-major fp16 x + coef tile."""
    x = np.asarray(x)
    alpha = np.asarray(alpha)
    assert x.shape == (B, T, D) and alpha.shape == (1, 1, D)
    coef = make_coef(alpha)
    xt = np.ascontiguousarray(x.transpose(0, 2, 1)).astype(np.float16)  # (B, D, T)
    # Per TC chunk: [NH evens | NH odds]
    xb = xt.reshape(B, D, NTC, NH, 2).transpose(0, 1, 2, 4, 3).reshape(B, D, T)
    xb = np.ascontiguousarray(xb)
    return [
        {"xt": np.ascontiguousarray(xb[i * BL : (i + 1) * BL]), "coef": coef}
        for i in range(NCORES)
    ]


def gather(results):
    """(NCORES, BL, D, T) fp16 blocked shards -> (B, T, D) f32.

    Per TC chunk the device wrote [NH evens | NH odds]; interleave back.
    """
    yt = np.concatenate([r["yt"] for r in results], axis=0)  # (B, D, T) blocked
    blk = yt.reshape(B, D, NTC, 2, NH)  # [..., 0, :] evens, [..., 1, :] odds
    nat = blk.transpose(0, 1, 2, 4, 3).reshape(B, D, T)  # interleave
    return np.ascontiguousarray(nat.transpose(0, 2, 1)).astype(np.float32)


def kernel(x, alpha):
    res = run_bass_kernel_spmd(
        _get_prog(), make_in_maps(x, alpha), core_ids=list(range(NCORES))
    )
    return gather(res.results)


# revision 12
# speedup vs baseline: 1.0046x; 1.0046x over previous
"""Exponential smoothing (per-channel EMA over time) on 8 Trainium2 cores.

  s_0 = x_0 ; s_t = a * x_t + (1 - a) * s_{t-1},  a = sigmoid(alpha)  (per channel)

Full shapes: x (16, 4096, 512) f32, alpha (1, 1, 512) f32 -> out (16, 4096, 512).

Design (trace-driven):
  * All HBM I/O is fp16, time-major per core, with each TC-chunk stored as
    blocked halves [even timesteps | odd timesteps]: the host preps the
    layout and upcasts/interleaves the result (pure layout transforms; the
    2e-2 global-rel-err budget dwarfs fp16's 2^-11 and the EMA is a convex
    combination). Halves DMA bytes, removes on-device transposes, and keeps
    every on-device operand contiguous. Per-channel parameter transforms
    (a, w, w^2, 1/a -- 512 elements) are host-precomputed into one (128,16)
    f32 tile.
  * The hardware scan (TensorTensorScanArith, vector engine only -- it does
    not compile for gpsimd) costs ~2.13 ns/elem/lane regardless of dtype.
    The kernel scans ONLY the odd timesteps (an EMA with decay w^2 over
    combined inputs g_i = w*x_{2i} + x_{2i+1}):
      - combine g: tensor engine, diag(w) @ x_even + I @ x_odd into PSUM
        (the scan reads data1 straight from PSUM).
      - odd scan: r_{2i+1} = w^2 * r_{2i-1} + g_i on the vector engine,
        scanning r = s/a (initial r_{-1} = x_0/a) so raw x is the scan
        input. The scan writes cols 1..NH of an [128, NH+1] tile whose col
        0 holds the initial; the fill's shifted operand is then contiguous.
      - even fill: r_{2i} = w * r_{2i-1} + x_{2i} on the tensor engine,
        accumulated IN PLACE over the chunk's g tile (its scan is done),
        halving PSUM pressure so g can quad-buffer.
      - evacuate+scale y = a * r: even halves (PSUM) on the scalar engine,
        odd halves (SBUF) on the vector engine's packed-fp16 fast path
        (~0.2 ns/elem; gpsimd elementwise ops crash the Q7 handler).
  * Chunks are processed in (tci, j, b) order and PAIRED over b: paired
    combines/fills batch 4 matmuls per stationary load, halving LDWEIGHTS
    churn, and combines run a pair ahead of fills so the tensor-engine
    stream stays dense (its clock ramps 1.2 -> 2.4 GHz only with ~3 us of
    continuous execution).
  * Loads ride the SP hardware-DGE queue, stores the Scalar hardware-DGE
    queue (the gpsimd software-DGE path sustains only ~170 GB/s); both
    spread across all 16 DMA engines (~44 us floor for 16.8 MB/core).
"""

from contextlib import ExitStack

import numpy as np

import concourse.tile as tile
from concourse import bacc, mybir
from concourse.bass_utils import run_bass_kernel_spmd
from concourse.masks import make_identity

B, T, D = 16, 4096, 512
NCORES = 8
BL = B // NCORES   # batches per core
P = 128            # partitions
TC = 2048          # time chunk per pipeline step
NH = TC // 2       # odd (= even) timesteps per chunk
ND = D // P        # channel chunks of 128
MM = 512           # max moving free dim per matmul
NTC = T // TC

FP32 = mybir.dt.float32
FP16 = mybir.dt.float16


def build_program(bl: int = BL, t: int = T) -> bacc.Bacc:
    """Build the per-core Bass program (same NEFF for all 8 cores)."""
    ntc = t // TC
    nc = bacc.Bacc(
        "TRN2",
        target_bir_lowering=False,
        debug=False,
        enable_asserts=False,
        num_devices=NCORES,
    )
    x = nc.dram_tensor("xt", (bl, D, t), FP16, kind="ExternalInput").ap()
    # Host-precomputed per-channel coefficients, partition-major:
    # col q*ND + j = quantity q for channel chunk j (q: 0=a, 1=w, 2=w^2, 3=1/a)
    coef = nc.dram_tensor("coef", (P, 4 * ND), FP32, kind="ExternalInput").ap()
    y = nc.dram_tensor("yt", (bl, D, t), FP16, kind="ExternalOutput").ap()

    with tile.TileContext(nc) as tc, ExitStack() as ctx:
        const_pool = ctx.enter_context(tc.tile_pool(name="const", bufs=1))
        x_pool = ctx.enter_context(tc.tile_pool(name="x", bufs=8))
        g_pool = ctx.enter_context(tc.tile_pool(name="g", bufs=4, space="PSUM"))
        r_pool = ctx.enter_context(tc.tile_pool(name="r", bufs=10))
        y_pool = ctx.enter_context(tc.tile_pool(name="y", bufs=6))

        # Identity first: gpsimd builds it while the coef DMA runs.
        ident = const_pool.tile([P, P], FP16)
        make_identity(nc, ident[:])

        coef_sb = const_pool.tile([P, 4 * ND], FP32)
        nc.sync.dma_start(coef_sb[:], coef[:, :])
        a_sb = coef_sb[:, 0 * ND : 1 * ND]
        w_sb = coef_sb[:, 1 * ND : 2 * ND]
        w2_sb = coef_sb[:, 2 * ND : 3 * ND]
        inv_a = coef_sb[:, 3 * ND : 4 * ND]

        diag_w = []
        for j in range(ND):
            dw = const_pool.tile([P, P], FP16, tag=f"dw{j}")
            nc.vector.tensor_scalar_mul(dw[:], ident[:], w_sb[:, j : j + 1])
            diag_w.append(dw)

        # Scan data0: w^2 broadcast along the time axis (full packed operand).
        ones = const_pool.tile([P, NH], FP16)
        nc.vector.memset(ones[:], 1.0)
        w2b = []
        for j in range(ND):
            wt = const_pool.tile([P, NH], FP16, tag=f"w2b{j}")
            nc.scalar.mul(wt[:], ones[:], w2_sb[:, j : j + 1])
            w2b.append(wt)

        # Chunks in (tci, j, b) order: consecutive pairs share diag_w[j].
        chunks = [
            (tci, b, j) for tci in range(ntc) for j in range(ND) for b in range(bl)
        ]
        n = len(chunks)
        npair = n // 2
        xcs, gps, ros = [None] * n, [None] * n, [None] * n
        r_prev = [[None] * ND for _ in range(bl)]

        def load_combine_pair(k):
            cs = (2 * k, 2 * k + 1)
            for c in cs:
                tci, b, j = chunks[c]
                t0 = tci * TC
                xc = x_pool.tile([P, TC], FP16, tag="x", name=f"x{c}")
                nc.sync.dma_start(xc[:], x[b, j * P : (j + 1) * P, t0 : t0 + TC])
                gps[c] = g_pool.tile([P, NH], FP32, tag="g", name=f"g{c}")
                xcs[c] = xc
            j = chunks[2 * k][2]
            # g = diag(w) @ x_even + I @ x_odd, batched per stationary.
            for c in cs:
                for h in range(NH // MM):
                    c0, c1 = h * MM, (h + 1) * MM
                    nc.tensor.matmul(
                        gps[c][:, c0:c1], diag_w[j][:], xcs[c][:, c0:c1],
                        start=True, stop=False,
                    )
            for c in cs:
                for h in range(NH // MM):
                    c0, c1 = h * MM, (h + 1) * MM
                    nc.tensor.matmul(
                        gps[c][:, c0:c1], ident[:], xcs[c][:, NH + c0 : NH + c1],
                        start=False, stop=True,
                    )

        def scan_chunk(c):
            tci, b, j = chunks[c]
            ro = r_pool.tile([P, NH + 1], FP16, tag="r", name=f"r{c}")
            if tci == 0:
                nc.vector.tensor_scalar_mul(
                    ro[:, 0:1], xcs[c][:, 0:1], inv_a[:, j : j + 1]
                )
            else:
                nc.vector.tensor_copy(ro[:, 0:1], r_prev[b][j][:, NH : NH + 1])
            nc.vector.tensor_tensor_scan(
                ro[:, 1 : NH + 1],
                w2b[j][:],
                gps[c][:],
                ro[:, 0:1],
                mybir.AluOpType.mult,
                mybir.AluOpType.add,
            )
            r_prev[b][j] = ros[c] = ro

        def fill_pair(k):
            # r_even = diag(w) @ ro_shift + I @ x_even, accumulated in place
            # over the pair's g tiles (their scans are done).
            cs = (2 * k, 2 * k + 1)
            j = chunks[2 * k][2]
            for c in cs:
                for h in range(NH // MM):
                    c0, c1 = h * MM, (h + 1) * MM
                    nc.tensor.matmul(
                        gps[c][:, c0:c1], diag_w[j][:], ros[c][:, c0:c1],
                        start=True, stop=False,
                    )
            for c in cs:
                for h in range(NH // MM):
                    c0, c1 = h * MM, (h + 1) * MM
                    nc.tensor.matmul(
                        gps[c][:, c0:c1], ident[:], xcs[c][:, c0:c1],
                        start=False, stop=True,
                    )

        def evac_store(c):
            tci, b, j = chunks[c]
            t0 = tci * TC
            yc = y_pool.tile([P, TC], FP16, tag="y")
            nc.scalar.mul(yc[:, 0:NH], gps[c][:], a_sb[:, j : j + 1])
            nc.vector.tensor_scalar_mul(
                yc[:, NH:TC], ros[c][:, 1 : NH + 1], a_sb[:, j : j + 1]
            )
            nc.scalar.dma_start(y[b, j * P : (j + 1) * P, t0 : t0 + TC], yc[:])
            xcs[c] = None  # release

        load_combine_pair(0)
        load_combine_pair(1)
        for k in range(npair):
            scan_chunk(2 * k)
            scan_chunk(2 * k + 1)
            fill_pair(k)
            if k + 2 < npair:
                load_combine_pair(k + 2)
            evac_store(2 * k)
            evac_store(2 * k + 1)

    nc.compile()
    return nc


_prog = None


def _get_prog():
    global _prog
    if _prog is None:
        _prog = build_program()
    return _prog


def make_coef(alpha):
    """Precompute per-channel (a, w, w^2, 1/a) packed as (128, 4*ND) f32."""
    al = np.asarray(alpha, dtype=np.float64).reshape(D)
    a = 1.0 / (1.0 + np.exp(-al))
    w = 1.0 - a
    quants = [a, w, w * w, 1.0 / a]
    out = np.empty((P, 4 * ND), dtype=np.float32)
    for q, v in enumerate(quants):
        # channel d = j*128 + p -> column q*ND + j, row p
        out[:, q * ND : (q + 1) * ND] = v.reshape(ND, P).T
    return out


def make_in_maps(x, alpha):
    """Per-core inputs: blocked-even/odd time-major fp16 x + coef tile."""
    x = np.asarray(x)
    alpha = np.asarray(alpha)
    assert x.shape == (B, T, D) and alpha.shape == (1, 1, D)
    coef = make_coef(alpha)
    xt = np.ascontiguousarray(x.transpose(0, 2, 1)).astype(np.float16)  # (B, D, T)
    # Per TC chunk: [NH evens | NH odds]
    xb = xt.reshape(B, D, NTC, NH, 2).transpose(0, 1, 2, 4, 3).reshape(B, D, T)
    xb = np.ascontiguousarray(xb)
    return [
        {"xt": np.ascontiguousarray(xb[i * BL : (i + 1) * BL]), "coef": coef}
        for i in range(NCORES)
    ]


def gather(results):
    """(NCORES, BL, D, T) fp16 blocked shards -> (B, T, D) f32.

    Per TC chunk the device wrote [NH evens | NH odds]; interleave back.
    """
    yt = np.concatenate([r["yt"] for r in results], axis=0)  # (B, D, T) blocked
    blk = yt.reshape(B, D, NTC, 2, NH)  # [..., 0, :] evens, [..., 1, :] odds
    nat = blk.transpose(0, 1, 2, 4, 3).reshape(B, D, T)  # interleave
    return np.ascontiguousarray(nat.transpose(0, 2, 1)).astype(np.float32)


def kernel(x, alpha):
    res = run_bass_kernel_spmd(
        _get_prog(), make_in_maps(x, alpha), core_ids=list(range(NCORES))
    )
    return gather(res.results)
